# revision 90
# baseline (speedup 1.0000x reference)
"""Multi-head self-attention (RoPE, causal) Trainium2 Bass kernel.

Sharding: tensor-parallel over heads. 16 heads / 8 cores = 2 heads per core.
Each core computes Q/K/V projections for its 2 heads, causal flash attention,
and a partial output projection against its 256-column slice of Wo. The host
sums the 8 partial [S, D] outputs.

All matmuls run in bf16 with fp32 PSUM accumulation. Softmax skips the max
subtraction (scores are O(1) for this problem family; exp stays in fp32
range), so the denominator comes free via a ones-column appended to V.
RoPE's even/odd interleave is folded into a row permutation of Wq/Wk, making
the on-device rotation a contiguous rotate-half.

Schedule: the softmax exp runs on the Activation engine at half the PE's
score-streaming rate, so attention is emitted as a software-pipelined
granule loop over k-tiles - score(j+2) -> exp(j+2) -> P@V(j) - with four
per-q-subtile PSUM accumulators (one bank each; accumulation groups are
bank-granular). Head-1's projections are paced into head-0's attention
phase as PE filler, and the output projection is paced into head-1's,
unlocked per finished m-tile. The P@V -> output-projection transpose runs
on the DMA xbar (dma_start_transpose), and all PSUM->SBUF copies run on
Pool/DVE so the Activation engine does nothing but exp during attention.
"""
import sys
from collections import deque
from contextlib import ExitStack

sys.path.insert(0, "/opt/trn_rl_repo")

import numpy as np
import ml_dtypes

import concourse.bass as bass  # noqa: F401  (registers AP machinery)
import concourse.tile as tile
from concourse import bacc, mybir
from concourse import bass_utils
from concourse.masks import make_identity

USE_DMA_TRANSPOSE = False

BF16 = ml_dtypes.bfloat16
S = 4096
D = 2048
DH = 128
N_CORES = 8
HPC = 2  # heads per core
N_KT = S // 128  # 32 k-tiles
QW = 512  # attention q-window
NW = S // QW  # 8 windows per head
PW2 = 256  # head-1 projection window (filler granularity)
INV_SQRT_DH = float(1.0 / np.sqrt(128.0))

_CACHE = {}


class PacedFiller:
    """Emits filler pieces (cols, fn) so cumulative filler PE-columns track
    the attention loop's cumulative PE-columns times `ratio` (plus a small
    front-load so early low-occupancy windows get extra PE work). Unlocked
    pieces become eligible only `delay` advance-calls after unlock, giving
    their upstream DMA (the oc transpose) time to land."""

    def __init__(self, ratio, front=0.0, delay=0):
        self.q = deque()
        self.locked = {}
        self.pending = deque()
        self.emitted = 0
        self.ratio = ratio
        self.front = front
        self.delay = delay

    def add(self, pieces):
        self.q.extend(pieces)

    def add_locked(self, key, pieces):
        self.locked[key] = pieces

    def unlock(self, key):
        if key in self.locked:
            self.pending.append([self.delay, self.locked.pop(key)])

    def _promote(self):
        for item in self.pending:
            item[0] -= 1
        while self.pending and self.pending[0][0] <= 0:
            self.q.extend(self.pending.popleft()[1])

    def advance(self, target_cols):
        self._promote()
        while self.q and self.emitted < target_cols * self.ratio + self.front:
            cols, fn = self.q.popleft()
            fn()
            self.emitted += cols

    def drain(self):
        while self.pending:
            self.q.extend(self.pending.popleft()[1])
        for key in list(self.locked):
            self.q.extend(self.locked.pop(key))
        while self.q:
            cols, fn = self.q.popleft()
            fn()
            self.emitted += cols


def _build(reps=1):
    fp32 = mybir.dt.float32
    bf16 = mybir.dt.bfloat16

    nc = bacc.Bacc("TRN2", target_bir_lowering=False, debug=False,
                   num_devices=N_CORES)
    xT_d = nc.dram_tensor("xT", [D, S], bf16, kind="ExternalInput").ap()
    wq_d = nc.dram_tensor("wqT", [D, HPC * DH], bf16, kind="ExternalInput").ap()
    wk_d = nc.dram_tensor("wkT", [D, HPC * DH], bf16, kind="ExternalInput").ap()
    wv_d = nc.dram_tensor("wvT", [D, HPC * DH], bf16, kind="ExternalInput").ap()
    wo_d = nc.dram_tensor("woT", [HPC * DH, D], bf16, kind="ExternalInput").ap()
    cos_d = nc.dram_tensor("cosF", [128, S], bf16, kind="ExternalInput").ap()
    sin_d = nc.dram_tensor("sinX", [128, S], bf16, kind="ExternalInput").ap()
    swp_d = nc.dram_tensor("swp", [128, 128], bf16, kind="ExternalInput").ap()
    mask_d = nc.dram_tensor("mask", [128, 128], bf16, kind="ExternalInput").ap()
    out_d = nc.dram_tensor("out", [S, D], bf16, kind="ExternalOutput").ap()

    xT_r = xT_d.rearrange("(t p) s -> p t s", p=128)   # [128, 16, S]
    wq_r = wq_d.rearrange("(t p) m -> p t m", p=128)   # [128, 16, 256]
    wk_r = wk_d.rearrange("(t p) m -> p t m", p=128)
    wv_r = wv_d.rearrange("(t p) m -> p t m", p=128)

    EXP = mybir.ActivationFunctionType.Exp

    def emit_body(tc):
        with tc.tile_pool(name="persist", bufs=1) as pp, \
             tc.tile_pool(name="ropet", bufs=2) as rtp, \
             tc.tile_pool(name="rswp", bufs=4) as rsp, \
             ExitStack() as stW:
            # ---- persistent tiles + initial loads
            qt = [pp.tile([128, S], bf16, tag=f"qt{h}", name=f"qt{h}")
                  for h in range(HPC)]
            kt = [pp.tile([128, S], bf16, tag=f"kt{h}", name=f"kt{h}")
                  for h in range(HPC)]
            v_sb = pp.tile([128, N_KT, 2 * (DH + 1)], bf16, tag="v",
                           name="v_sb")
            # per-m-tile oc tiles: coarser granularity makes each transpose
            # write WAR-depend on unrelated C-matmul reads of the same tile
            oc = [[pp.tile([128, 128], bf16, tag=f"oc{h}m{m}",
                           name=f"oc{h}m{m}") for m in range(N_KT)]
                  for h in range(HPC)]
            cos_sb = pp.tile([128, S], bf16, tag="cos", name="cos_sb")
            sin_sb = pp.tile([128, S], bf16, tag="sin", name="sin_sb")
            mask_sb = pp.tile([128, 128], bf16, tag="mask", name="mask_sb")
            swp_sb = pp.tile([128, 128], bf16, tag="swp", name="swp_sb")
            # startup-critical loads go on sync (ordered); non-critical loads
            # are deferred into the A-window loop below so they don't contend
            # for DMA bandwidth during the DMA-bound first window.
            nc.gpsimd.dma_start(out=mask_sb, in_=mask_d)
            wo_sb = [pp.tile([128, D], bf16, tag=f"wo{t}", name=f"wo{t}")
                     for t in range(HPC)]
            # ones columns of the augmented V (constant across the body)
            nc.vector.memset(v_sb[:, :, DH:DH + 1], 1.0)
            nc.vector.memset(v_sb[:, :, 2 * DH + 1:2 * DH + 2], 1.0)

            ident_sb = None
            if not USE_DMA_TRANSPOSE:
                ident_sb = pp.tile([128, 128], bf16, tag="ident",
                                   name="ident_sb")
                make_identity(nc, ident_sb)

            wqkp = stW.enter_context(tc.tile_pool(name="wqk", bufs=1))
            wq_sb = wqkp.tile([128, 16, HPC * DH], bf16, tag="wq", name="wq_sb")
            wk_sb = wqkp.tile([128, 16, HPC * DH], bf16, tag="wk", name="wk_sb")

            # rope's half-swap runs as a partition-rotate SBUF->SBUF DMA
            # instead of a PE matmul: saves the PE columns, removes the
            # swp_sb weight-source switch between projection runs, frees the
            # swap's PSUM slot, and lets the sin-multiply read bf16 SBUF.
            def rope_swap_dma(dest, sl, pw, eng):
                sw = rsp.tile([128, pw], bf16, tag="sw", name="sw",
                              padded_shape=[128, 512])
                eng.dma_start(out=sw[0:64, :], in_=dest[64:128, sl])
                eng.dma_start(out=sw[64:128, :], in_=dest[0:64, sl])
                return sw

            def rope_mul(dest, sl, pw, sw):
                # dest[:, sl] = dest*cosF + halfswap(dest)*[-sin;sin]
                dsl = dest[:, sl]
                m1 = rtp.tile([128, pw], bf16, tag="m1", name="m1",
                              padded_shape=[128, 512])
                m2 = rtp.tile([128, pw], bf16, tag="m2", name="m2",
                              padded_shape=[128, 512])
                nc.vector.tensor_mul(m1, dsl, cos_sb[:, sl])
                nc.vector.tensor_mul(m2, sw, sin_sb[:, sl])
                nc.vector.tensor_add(dsl, m1, m2)

            # ---------------- Phase A: head-0 Q/K + all V ----------------
            stA = ExitStack()
            # psA holds 4 tiles per window (Q ps, Q swap, K ps, K swap):
            # bufs=4 matches the rotation period so a new window's Q ps only
            # WARs the fast copy-drained slot, not the slow rope-drained one
            psA = stA.enter_context(
                tc.tile_pool(name="psA", bufs=4, space="PSUM"))
            psV = stA.enter_context(
                tc.tile_pool(name="psV", bufs=3, space="PSUM"))
            wvp = stA.enter_context(tc.tile_pool(name="wvp", bufs=1))
            xw1p = stA.enter_context(tc.tile_pool(name="xw1", bufs=3))
            wv_sb = wvp.tile([128, 16, HPC * DH], bf16, tag="wv", name="wv_sb")
            # window 0 is DMA-critical: interleave weight/x chunks on one
            # ordered queue so matmuls start as data lands; only head-0
            # weight halves are needed in phase A
            xw0 = xw1p.tile([128, 16, 512], bf16, tag="xw", name="xw")
            for ch in range(4):
                nc.sync.dma_start(out=wq_sb[:, 4 * ch:4 * ch + 4, 0:DH],
                                  in_=wq_r[:, 4 * ch:4 * ch + 4, 0:DH])
                nc.sync.dma_start(out=xw0[:, 4 * ch:4 * ch + 4, :],
                                  in_=xT_r[:, 4 * ch:4 * ch + 4, 0:512])
                if ch == 0:
                    nc.sync.dma_start(out=swp_sb, in_=swp_d)
            nc.sync.dma_start(out=wk_sb[:, :, 0:DH], in_=wk_r[:, :, 0:DH])
            nc.sync.dma_start(out=cos_sb[:, 0:512], in_=cos_d[:, 0:512])
            nc.sync.dma_start(out=sin_sb[:, 0:512], in_=sin_d[:, 0:512])
            nc.sync.dma_start(out=wv_sb, in_=wv_r)
            xw1 = xw1p.tile([128, 16, 512], bf16, tag="xw", name="xw")
            nc.sync.dma_start(out=xw1, in_=xT_r[:, :, 512:1024])
            # deferred non-critical loads, emitted into the in-order engine
            # queues between windows so they miss the DMA-bound startup but
            # meet their deadlines (cos/sin chunk before window w's rope,
            # h1 weights for B0, wo for B1)
            deferred = {
                0: [(cos_sb[:, 512:1024], cos_d[:, 512:1024]),
                    (sin_sb[:, 512:1024], sin_d[:, 512:1024])],
                1: [(cos_sb[:, 1024:2048], cos_d[:, 1024:2048]),
                    (sin_sb[:, 1024:2048], sin_d[:, 1024:2048])],
                2: [(cos_sb[:, 2048:], cos_d[:, 2048:])],
                3: [(sin_sb[:, 2048:], sin_d[:, 2048:])],
                4: [(wq_sb[:, :, DH:2 * DH], wq_r[:, :, DH:2 * DH]),
                    (wk_sb[:, :, DH:2 * DH], wk_r[:, :, DH:2 * DH])],
                5: [(wo_sb[0], wo_d[0:128, :]), (wo_sb[1], wo_d[128:256, :])],
            }
            xw_tiles_a = [xw0, xw1]
            for w in range(8):
                sl = slice(512 * w, 512 * w + 512)
                xw = xw_tiles_a[w]
                if w + 2 < 8:
                    # prefetch window w+2's x before this window's copies
                    # enter the queues
                    nxt = xw1p.tile([128, 16, 512], bf16, tag="xw", name="xw")
                    eng = nc.sync if w % 2 == 0 else nc.gpsimd
                    eng.dma_start(out=nxt,
                                  in_=xT_r[:, :, 512 * (w + 2):512 * (w + 3)])
                    xw_tiles_a.append(nxt)
                for dst, src in deferred.get(w, []):
                    nc.gpsimd.dma_start(out=dst, in_=src)
                for wsb, dest in ((wq_sb, qt[0]), (wk_sb, kt[0])):
                    ps = psA.tile([128, 512], fp32, tag="qk", name="ps")
                    for t in range(16):
                        nc.tensor.matmul(ps, wsb[:, t, 0:DH], xw[:, t, :],
                                         start=(t == 0), stop=(t == 15))
                    # PSUM drains must be ACT/DVE (Pool can't read PSUM);
                    # ACT is idle in phase A
                    nc.scalar.copy(out=dest[:, sl], in_=ps)
                    sw = rope_swap_dma(dest, sl, 512,
                                       nc.sync if w % 2 == 0 else nc.gpsimd)
                    rope_mul(dest, sl, 512, sw)
                for sub in range(4):
                    st = 4 * w + sub
                    ssl = slice(sub * 128, (sub + 1) * 128)
                    pv = psV.tile([128, HPC * DH], fp32, tag="v", name="pv")
                    for t in range(16):
                        nc.tensor.matmul(pv, xw[:, t, ssl], wv_sb[:, t, :],
                                         start=(t == 0), stop=(t == 15))
                    vt = v_sb[:, st, :]
                    nc.scalar.copy(out=vt[:, 0:DH], in_=pv[:, 0:DH])
                    nc.scalar.copy(out=vt[:, DH + 1:2 * DH + 1],
                                   in_=pv[:, DH:2 * DH])
            stA.close()

            # ---------------- attention phase machinery ----------------
            gran = [(w, j) for w in range(NW) for j in range(4 * w + 4)]

            def gcols(w, j):
                c = j - 4 * w
                lo = max(c, 0) * 128
                return (512 - lo) + (4 - max(c, 0)) * 129

            cum = np.cumsum([gcols(w, j) for (w, j) in gran])
            B_TOTAL = float(cum[-1])

            def b_phase(h, pssc_bufs, filler, pool_box=None):
                # Window-pipelined: while window w's P@V runs as four long
                # same-bank accumulation runs (few PSUM bank switches, which
                # cost real time on HW), window w+1's scores stream to the
                # Activation engine, giving exp a full window of slack.
                stB = ExitStack()
                pssc = stB.enter_context(
                    tc.tile_pool(name=f"sc{h}", bufs=pssc_bufs, space="PSUM"))
                if pool_box is not None:
                    pool_box.append(pssc)
                pstr = None
                if not USE_DMA_TRANSPOSE:
                    pstr = stB.enter_context(
                        tc.tile_pool(name=f"tr{h}", bufs=1, space="PSUM"))
                paug = stB.enter_context(
                    tc.tile_pool(name=f"aug{h}", bufs=1, space="PSUM"))
                ptp = stB.enter_context(tc.tile_pool(name=f"pt{h}", bufs=2))
                bst = stB.enter_context(tc.tile_pool(name=f"bst{h}", bufs=3))
                vsl = slice(h * (DH + 1), (h + 1) * (DH + 1))
                cur_pts = {}
                nxt_pts = {}
                cols = [0.0]

                def produce_one(w, j):
                    # score + exp + mask for (window w, k-tile j)
                    c = j - 4 * w
                    lo = max(c, 0) * 128
                    q0 = 512 * w
                    sc = pssc.tile([128, 512], fp32, tag="sc", name="sc")
                    nc.tensor.matmul(sc[:, lo:],
                                     kt[h][:, j * 128:(j + 1) * 128],
                                     qt[h][:, q0 + lo:q0 + 512],
                                     start=True, stop=True)
                    pt = ptp.tile([128, 512], bf16, tag=f"pt{j}",
                                  name=f"pt{j}")
                    nc.scalar.activation(pt[:, lo:], sc[:, lo:], EXP,
                                         scale=INV_SQRT_DH)
                    if c >= 0:
                        nc.gpsimd.tensor_mul(pt[:, lo:lo + 128],
                                             pt[:, lo:lo + 128], mask_sb)
                    nxt_pts[j] = pt
                    cols[0] += 512 - lo

                def pv_run(w, i):
                    # one long accumulation run into a single PSUM bank
                    m = 4 * w + i
                    aug = paug.tile([128, DH + 1], fp32, tag=f"aug{i}",
                                    name=f"aug{i}")
                    for j in range(m + 1):
                        nc.tensor.matmul(aug,
                                         cur_pts[j][:, i * 128:(i + 1) * 128],
                                         v_sb[:, j, vsl],
                                         start=(j == 0), stop=(j == m))
                    cols[0] += (m + 1) * 129
                    rc = bst.tile([128, 1], fp32, tag="rc", name="rc")
                    nc.vector.reciprocal(rc, aug[:, DH:DH + 1])
                    stg = bst.tile([128, 128], bf16, tag="st", name="stg")
                    # per-partition scale on ACT keeps the aug drain off
                    # DVE's so-copy-loaded queue
                    nc.scalar.mul(stg, aug[:, 0:DH], rc)
                    if USE_DMA_TRANSPOSE:
                        nc.sync.dma_start_transpose(oc[h][m], stg)
                    else:
                        tr = pstr.tile([128, 128], bf16, tag="tr", name="tr")
                        nc.tensor.transpose(tr, stg, ident_sb)
                        nc.vector.tensor_copy(oc[h][m], tr)
                    filler.unlock(m)

                # prologue: window 0's scores
                for j in range(4):
                    produce_one(0, j)
                    filler.advance(cols[0])
                cur_pts, nxt_pts = nxt_pts, cur_pts
                for w in range(NW):
                    prod = ([(w + 1, j) for j in range(4 * (w + 1) + 4)]
                            if w + 1 < NW else [])
                    # spread next window's production across this window's
                    # four P@V runs
                    chunk = (len(prod) + 3) // 4 if prod else 0
                    for i in range(4):
                        sub = prod[i * chunk:(i + 1) * chunk]
                        for k, (pw_, pj) in enumerate(sub):
                            produce_one(pw_, pj)
                            # filler every 2nd score: enough to cover the
                            # 2-deep score rotation without doubling the
                            # number of stream-switch boundaries
                            if k % 2 == 1 or k == len(sub) - 1:
                                filler.advance(cols[0])
                        pv_run(w, i)
                        filler.advance(cols[0])
                    cur_pts, nxt_pts = nxt_pts, cur_pts
                filler.drain()
                stB.close()

            # ------- Phase B0: head-0 attention + head-1 Q/K filler -------
            stP = ExitStack()
            b0_pool_box = []
            psP = stP.enter_context(
                tc.tile_pool(name="psP", bufs=1, space="PSUM"))
            xw2p = stP.enter_context(tc.tile_pool(name="xw2", bufs=3))
            xw_tiles = {}
            sw_state = {}

            def dma_xw2(q):
                xw = xw2p.tile([128, 16, PW2], bf16, tag="xw2", name="xw2")
                nc.sync.dma_start(out=xw, in_=xT_r[:, :, q * PW2:(q + 1) * PW2])
                xw_tiles[q] = xw

            def mk_proj_ab(q, di, wsb, dest):
                # one contiguous 16-matmul run + copy: fewer stream-switch
                # boundaries than two half-pieces
                def fn():
                    if di == 0 and q + 2 < 16:
                        dma_xw2(q + 2)
                    ps = psP.tile([128, PW2], fp32, tag="pj", name="pj")
                    xw = xw_tiles[q]
                    for t in range(16):
                        nc.tensor.matmul(ps, wsb[:, t, DH:2 * DH], xw[:, t, :],
                                         start=(t == 0), stop=(t == 15))
                    sl = slice(q * PW2, (q + 1) * PW2)
                    nc.vector.tensor_copy(dest[:, sl], ps)
                    # dispatch the swap DMA now so its latency is hidden
                    # before the rope piece's DVE ops need the data
                    sw_state[(q, di)] = rope_swap_dma(dest, sl, PW2, nc.sync)
                return fn

            def mk_proj_r(q, di, dest):
                def fn():
                    rope_mul(dest, slice(q * PW2, (q + 1) * PW2), PW2,
                             sw_state.pop((q, di)))
                return fn

            proj_pieces = [(1, lambda: dma_xw2(0)), (1, lambda: dma_xw2(1))]
            for q in range(16):
                for di, (wsb, dest) in enumerate(((wq_sb, qt[1]),
                                                  (wk_sb, kt[1]))):
                    proj_pieces.append((4096, mk_proj_ab(q, di, wsb, dest)))
                    proj_pieces.append((PW2, mk_proj_r(q, di, dest)))
            PROJ_TOTAL = float(sum(c for c, _ in proj_pieces))
            f0 = PacedFiller(PROJ_TOTAL / B_TOTAL)
            f0.add(proj_pieces)
            b_phase(0, 3 if USE_DMA_TRANSPOSE else 2, f0,
                    pool_box=b0_pool_box)
            stP.close()

            # ------- Phase B1: head-1 attention + output-proj filler -------
            stC = ExitStack()
            cst = stC.enter_context(tc.tile_pool(name="cst", bufs=6))
            so_state = {}
            c_pool_box = []

            def mk_c2(m, half):
                # two n-blocks per piece: one contiguous 4-matmul run with
                # its two copies, halving C's stream-switch boundaries
                def fn():
                    if half == 0:
                        so_state[m] = cst.tile([128, D], bf16, tag="so",
                                               name="so")
                    so = so_state[m]
                    msl = slice(m * 128, (m + 1) * 128)
                    for nw in (2 * half, 2 * half + 1):
                        ps = c_pool_box[0].tile([128, 512], fp32, tag="sc",
                                                name="cps")
                        nsl = slice(nw * 512, (nw + 1) * 512)
                        for t in range(HPC):
                            nc.tensor.matmul(ps, oc[t][m], wo_sb[t][:, nsl],
                                             start=(t == 0),
                                             stop=(t == HPC - 1))
                        nc.vector.tensor_copy(so[:, nsl], ps)
                        if m == 31:
                            # last m-tile: per-block DMA shortens the drain
                            nc.sync.dma_start(out=out_d[msl, nsl],
                                              in_=so[:, nsl])
                    if half == 1:
                        if m == 31:
                            so_state.pop(m)
                        else:
                            nc.sync.dma_start(out=out_d[msl, :],
                                              in_=so_state.pop(m))
                return fn

            C_TOTAL = float(32 * 4 * 1024)
            # 15% pacing lead: C pieces are unlock-gated anyway, so a lead
            # just drains each window's C promptly and shrinks the tail;
            # delay=2 granules keeps C matmuls behind the oc transposes
            f1 = PacedFiller(C_TOTAL / B_TOTAL * 1.15, delay=2)
            for m in range(32):
                f1.add_locked(m, [(2048, mk_c2(m, half))
                                  for half in range(2)])

            # C's matmul psum tiles share the b_phase score rotation (tag
            # "sc"); the pool is exposed via c_pool_box before any filler
            # piece runs.
            b_phase(1, 4 if USE_DMA_TRANSPOSE else 3, f1,
                    pool_box=c_pool_box)
            stC.close()

    with tile.TileContext(nc) as tc:
        for _ in range(reps):
            emit_body(tc)

    nc.compile()
    return nc


def _host_prep(inputs):
    x = np.ascontiguousarray(np.asarray(inputs["x"], dtype=np.float32)[0])  # [S, D]
    tp = np.asarray(inputs["token_positions"]).reshape(-1)[:S]
    Wq = np.asarray(inputs["Wq"], dtype=np.float32)
    Wk = np.asarray(inputs["Wk"], dtype=np.float32)
    Wv = np.asarray(inputs["Wv"], dtype=np.float32)
    Wo = np.asarray(inputs["Wo"], dtype=np.float32)

    xT = np.ascontiguousarray(x.T).astype(BF16)  # [D, S]

    # f32 RoPE tables, replicated across the two 64-row halves
    inv_freq = (10000.0 ** (-np.arange(0, DH, 2, dtype=np.float32) / DH)
                ).astype(np.float32)
    ang = tp.astype(np.float32)[:, None] * inv_freq[None, :]  # [S, 64] f32
    cos = np.cos(ang).astype(np.float32).T  # [64, S]
    sin = np.sin(ang).astype(np.float32).T
    cosF = np.concatenate([cos, cos], axis=0).astype(BF16)  # [128, S]
    sinX = np.concatenate([-sin, sin], axis=0).astype(BF16)
    # half-swap permutation as a matmul lhsT: out[m] = in[(m+64) % 128]
    swp = np.zeros((128, 128), dtype=np.float32)
    swp[np.arange(128), (np.arange(128) + 64) % 128] = 1.0
    swp = swp.astype(BF16)

    # causal mask in scores^T layout: valid iff k <= q  ->  upper triangular
    mask = np.triu(np.ones((128, 128), dtype=np.float32)).astype(BF16)

    perm = np.concatenate([np.arange(0, DH, 2), np.arange(1, DH, 2)])
    in_maps = []
    for c in range(N_CORES):
        rows = slice(c * HPC * DH, (c + 1) * HPC * DH)
        wq_blk = Wq[rows].reshape(HPC, DH, D)[:, perm].reshape(HPC * DH, D)
        wk_blk = Wk[rows].reshape(HPC, DH, D)[:, perm].reshape(HPC * DH, D)
        wv_blk = Wv[rows]
        in_maps.append({
            "xT": xT,
            "wqT": np.ascontiguousarray(wq_blk.T).astype(BF16),
            "wkT": np.ascontiguousarray(wk_blk.T).astype(BF16),
            "wvT": np.ascontiguousarray(wv_blk.T).astype(BF16),
            "woT": np.ascontiguousarray(Wo[:, rows].T).astype(BF16),
            "cosF": cosF,
            "sinX": sinX,
            "swp": swp,
            "mask": mask,
        })
    return in_maps


def get_compiled():
    if "nc" not in _CACHE:
        _CACHE["nc"] = _build()
    return _CACHE["nc"]


def kernel(**inputs):
    nc = get_compiled()
    in_maps = _host_prep(inputs)
    res = bass_utils.run_bass_kernel_spmd(
        nc, in_maps, core_ids=list(range(N_CORES)))
    y = np.zeros((S, D), dtype=np.float32)
    for c in range(N_CORES):
        y += res.results[c]["out"].astype(np.float32)
    return y.reshape(1, S, D)
